# revision 1
# baseline (speedup 1.0000x reference)
"""DGL capsule routing layer on 8 trn2 NeuronCores (Bass/Tile) — v3.

Math per iteration (b0 = 0):
    c = softmax(b, axis=out); s = einsum('io,iof->of', c, uh)
    v = squash(s); b += einsum('iof,of->io', uh, v)
Output: final v [OUT, F].

b_t = uh . w_{t-1} with w = cumulative v, so b is recomputed per pass.

v3 layout: uh cached in SBUF as bf16 with CHUNK-MAJOR f-outer columns:
flat col g = q*4096 + f*256 + o_l  (o = q*256 + o_l, q in 0..4).
Each pass-1 staging chunk (o-range q) converts into one CONTIGUOUS
4096-col span of the cache, so tile subtile-deps are exact and the pass-1
s matmuls (one [1,256] PSUM segment-group of 4 block-matmuls per (q,f))
start as soon as each chunk-set lands instead of after the whole load.
s / b / e / pt / AR all use this chunk-major flat order consistently:
  * AR payload ar[g] = s in chunk-major order, bf16, 32 KiB.
  * post-AR p-major [128,128] tiles: p = q*32 + f*2 + hi, free = lo
    (o = q*256 + hi*128 + lo); squash cross-partition f-sum and sc
    re-broadcast are one-hot PE matmuls (oh1: m=(p//32)*2+p%2,
    oh2: p'=(m//32)*2+m%2).
  * w accumulates bf16 in the same p-major tile; p-major flat IS the
    chunk-major flat, so the DRAM bounce + partition-stride-0 broadcast
    into w_fo line up with the uh cache for the next pass's tm mul.
Passes >= 2 per 128-i block: tm = uh*w as two f-half TT muls (strided
[p,4,2048] views, 2x_1p), b = in-place halving tree over f, all on DVE
(GpSimd mishandles strided 3D views and is 2.5x slower anyway), e =
exp(b) on ACT with fused denominator, rinv folded into the PE
stationary (bf16). s partials: per f-PAIR, pt2 = e*uh (TT, [p,(j,o)]
flat so each plane stays o-contiguous), two [1,512] window matmuls per
plane PSUM-accumulating over the 4 i-blocks; squash uses a Newton-free
sqrt via ACT exp(0.5*ln).

Measured 495275-496331 ns on 8 cores, rel err 5.39e-3 (budget 2e-2).
Tried and REGRESSED on HW (do not retry blindly): GpSimd tree offload
(wrong + slow), quarter-granular w-broadcast (+70us), all-DVE bit-trick
rsqrt squash (+51us; 14 serial tiny DVE ops lose to 2 ACT table loads).
Open frontier: chain/plane software-pipelining through shared PSUM
(blocked by psum capacity 3x[1,1024] + sc-pool rotation hazards), o-half
boundary pipelining (blocked by flush-count doubling), AR1 skew (~22us,
cross-core DMA jitter, unfixable in SPMD).
Front-end-verified for future use: partition-dim split+slice APs lower
(enables f-half-subset w_dram writes to gate each broadcast half ~2us
earlier), and collective_compute accepts strided APs (enables f-half AR
splitting; hardware behavior untested -- validate before trusting).
"""

import numpy as np
from contextlib import ExitStack

import concourse.bass as bass
import concourse.mybir as mybir
import concourse.tile as tile
from concourse import bacc
from concourse import bass_utils

F32 = mybir.dt.float32
BF16 = mybir.dt.bfloat16
AF = mybir.ActivationFunctionType
AO = mybir.AluOpType

IN_NODES, OUT_NODES, F_SIZE = 4096, 1024, 16
CORES = 8
I_LOC = IN_NODES // CORES          # 512 in-nodes per core
ROW = OUT_NODES * F_SIZE           # 16384 values per in-node row
P = 128
NBLK = I_LOC // P                  # 4 i-blocks per core
QT = 4096                          # staging chunk = cache chunk (cols)
NQT = ROW // QT                    # 4 chunks
O = OUT_NODES
H = ROW // 2                       # 8192
SEG = 256                          # o-cols per (q,f) cache segment


def _body(nc, tc, uh, v_out, R, rg):
    uh_t = uh.rearrange("(n p) r -> n p r", p=P)   # [NBLK, 128, 16384] f32

    with ExitStack() as ctx:
        persist = ctx.enter_context(tc.tile_pool(name="persist", bufs=1))
        scp = ctx.enter_context(tc.tile_pool(name="scp", bufs=2))
        smp = ctx.enter_context(tc.tile_pool(name="smp", bufs=1))
        psp = ctx.enter_context(tc.tile_pool(name="psp", bufs=3, space="PSUM"))
        dram = ctx.enter_context(tc.tile_pool(name="dram", bufs=2, space="DRAM"))

        # --- persistent tiles -------------------------------------------
        uhb = [persist.tile([P, ROW], BF16, name=f"uhb{k}", tag=f"uhb{k}")
               for k in range(NBLK)]
        w_fo = None
        if R > 1:
            w_fo = persist.tile([P, ROW], BF16, name="w_fo")
        c0 = persist.tile([P, 1], BF16, name="c0")
        nc.vector.memset(c0, 1.0 / OUT_NODES)
        # one-hot stationaries for squash in the chunk-major p-mapping:
        # p = q*32 + f*2 + hi  ->  m = q*2 + hi = (p//32)*2 + p%2
        pidx = np.arange(P)
        m_of_p = (pidx // 32) * 2 + (pidx % 2)
        oh1_d = nc.inline_tensor(
            (m_of_p[:, None] == np.arange(8)[None, :]).astype('bfloat16'),
            name="oh1d")
        oh2_d = nc.inline_tensor(
            (np.arange(8)[:, None] == m_of_p[None, :]).astype('bfloat16'),
            name="oh2d")
        oh1 = persist.tile([P, 8], BF16, name="oh1")
        nc.sync.dma_start(oh1, oh1_d.ap())
        oh2 = persist.tile([8, P], BF16, name="oh2")
        nc.sync.dma_start(oh2, oh2_d.ap())

        w_acc_prev = None

        def fhalf(tile_, h):
            # strided f-half view: [p, 4 chunks, 2048] (f<8 or f>=8)
            return tile_.rearrange("p (q c) -> p q c", c=QT)[
                :, :, h * (QT // 2):(h + 1) * (QT // 2)]

        for t in range(1, R + 1):
            ar_in = dram.tile([ROW], BF16, tag="ar_in")
            ar_v = ar_in.rearrange("(q f c) -> q f c", q=NQT, f=F_SIZE)
            if t == 1:
                # ---- pass 1: stream, convert, and matmul per chunk-set ----
                for q in range(NQT):
                    for blk in range(NBLK):
                        st = scp.tile([P, QT], F32, tag="sc", name="st")
                        nc.sync.dma_start(
                            st, uh_t[blk, :, q * QT:(q + 1) * QT])
                        dst = uhb[blk][:, q * QT:(q + 1) * QT].rearrange(
                            "p (f c) -> p f c", f=F_SIZE)
                        nc.vector.tensor_copy(
                            dst, st.rearrange("p (o f) -> p f o", f=F_SIZE))
                    # per f-quad: one [1,1024] psum, 4 segment-groups
                    for fq in range(F_SIZE // 4):
                        ps = psp.tile([1, O], F32, tag="ps1", name="ps")
                        for j in range(4):
                            f = fq * 4 + j
                            for blk in range(NBLK):
                                nc.tensor.matmul(
                                    ps[:, j * SEG:(j + 1) * SEG], c0,
                                    uhb[blk][:, q * QT + f * SEG:
                                             q * QT + (f + 1) * SEG],
                                    start=(blk == 0), stop=(blk == NBLK - 1),
                                    skip_group_check=True)
                        fl = smp.tile([1, O], BF16, tag="bfl", bufs=2,
                                      name="fl")
                        nc.scalar.copy(fl, ps)
                        # quad spans contiguous chunk-major cols
                        nc.sync.dma_start(
                            ar_in[q * QT + fq * O:q * QT + (fq + 1) * O], fl)
                rbs = [c0] * NBLK
                e2s = None
            else:
                # ---- passes >= 2: b, e, rinv per block from SBUF cache ----
                rbs, e2s = [], []
                for blk in range(NBLK):
                    tmA = scp.tile([P, H], BF16, tag="sc", name="tmA")
                    tmB = scp.tile([P, H], BF16, tag="sc", name="tmB")
                    tmAv = tmA.rearrange("p (q c) -> p q c", c=QT // 2)
                    tmBv = tmB.rearrange("p (q c) -> p q c", c=QT // 2)
                    nc.vector.tensor_mul(tmAv, fhalf(uhb[blk], 0),
                                         fhalf(w_fo, 0))
                    nc.vector.tensor_mul(tmBv, fhalf(uhb[blk], 1),
                                         fhalf(w_fo, 1))
                    # halving tree over f (in-place, all on DVE)
                    nc.vector.tensor_add(tmA, tmA, tmB)
                    nc.vector.tensor_add(
                        tmAv[:, :, 0:1024], tmAv[:, :, 0:1024],
                        tmAv[:, :, 1024:2048])
                    nc.vector.tensor_add(
                        tmAv[:, :, 0:512], tmAv[:, :, 0:512],
                        tmAv[:, :, 512:1024])
                    b = smp.tile([P, O], BF16, tag="bfl", bufs=2,
                                 name="b")
                    nc.vector.tensor_add(
                        b.rearrange("p (q c) -> p q c", c=SEG),
                        tmAv[:, :, 0:SEG], tmAv[:, :, SEG:2 * SEG])
                    e2 = smp.tile([P, O], BF16, tag=f"e2_{blk}", name="e2")
                    den = smp.tile([P, 1], F32, tag="den", name="den")
                    nc.scalar.activation(e2, b, AF.Exp, accum_out=den)
                    rinv = smp.tile([P, 1], F32, tag="rinv", name="rinv")
                    nc.vector.reciprocal(rinv, den)
                    rb = smp.tile([P, 1], BF16, tag=f"rb_{blk}", name="rb")
                    nc.vector.tensor_copy(rb, rinv)
                    rbs.append(rb)
                    e2s.append(e2)

                # ---- s partials: per f-plane, PSUM-accumulate over blocks
                for fp in range(F_SIZE // 2):
                    pss = [psp.tile([1, O], F32, tag="ps1", name="ps",
                                    padded_shape=[P, O]) for _ in range(2)]
                    for blk in range(NBLK):
                        # pt2 flat [p, (j, o)]: both planes of the pair,
                        # o-contiguous per plane for the 512-wide windows
                        pt2 = scp.tile([P, 2 * O], BF16, tag="sc", name="pt2")
                        nc.vector.tensor_mul(
                            pt2.rearrange("p (j q c) -> p q j c",
                                          j=2, c=SEG),
                            uhb[blk].rearrange("p (q c) -> p q c", c=QT)[
                                :, :, 2 * fp * SEG:(2 * fp + 2) * SEG]
                            .rearrange("p q (j c) -> p q j c", c=SEG),
                            e2s[blk].rearrange("p (q c) -> p q c", c=SEG)[
                                :, :, None, :].broadcast_to([P, NQT, 2, SEG]))
                        for j in range(2):
                            for w0 in (0, 512):
                                nc.tensor.matmul(
                                    pss[j][:, w0:w0 + 512], rbs[blk],
                                    pt2[:, j * O + w0:j * O + w0 + 512],
                                    start=(blk == 0), stop=(blk == NBLK - 1),
                                    skip_group_check=True)
                    for j in range(2):
                        fl = smp.tile([1, O], BF16, tag="bfl", bufs=2,
                                      name="fl")
                        nc.scalar.copy(fl, pss[j])
                        # s[f, o] o-ordered -> chunk-major ar positions
                        nc.sync.dma_start(ar_v[:, 2 * fp + j], fl)

            ar_out = dram.tile([ROW], BF16, tag="ar_out")
            nc.gpsimd.collective_compute(
                "AllReduce", AO.add, replica_groups=rg,
                ins=[ar_in.opt()], outs=[ar_out.opt()],
            )

            # ---- squash in p-major layout: p = q*32+f*2+hi, free = lo ----
            sld = smp.tile([P, P], BF16, tag="sld", name="sld")
            nc.sync.dma_start(sld, ar_out.rearrange("(p q) -> p q", p=P))
            ssq = smp.tile([P, P], BF16, tag="ssq", name="ssq")
            nc.vector.tensor_mul(ssq, sld, sld)
            sqps = psp.tile([8, P], F32, tag="sqps", bufs=1, name="sqps")
            nc.tensor.matmul(sqps, oh1, ssq, start=True, stop=True,
                             skip_group_check=True)
            sq = smp.tile([8, P], BF16, tag="sq", name="sq")
            nc.scalar.copy(sq, sqps)
            # sqrt(sq) via exp(0.5*ln) + one Newton step (exp/ln table set)
            lnq = smp.tile([8, P], BF16, tag="lnq", name="lnq")
            nc.scalar.activation(lnq, sq, AF.Ln)
            y = smp.tile([8, P], BF16, tag="y", name="y")
            nc.scalar.activation(y, lnq, AF.Exp, scale=0.5)
            d1 = smp.tile([8, P], BF16, tag="t1", name="d1")
            nc.vector.tensor_scalar(d1, sq, 1.0, None, AO.add)
            rd = smp.tile([8, P], BF16, tag="ry", name="rd")
            with nc.allow_low_precision(reason="bf16 squash chain"):
                nc.vector.reciprocal(rd, d1)       # rd = 1/(1+sq)
            sc = smp.tile([8, P], BF16, tag="sq", name="sc")
            nc.vector.tensor_mul(sc, y, rd)        # sqrt(sq)/(1+sq)
            srps = psp.tile([P, P], F32, tag="srps", bufs=1, name="srps")
            nc.tensor.matmul(srps, oh2, sc, start=True, stop=True,
                             skip_group_check=True)
            v_sb = smp.tile([P, P], BF16, tag="v_sb", name="v_sb")
            nc.vector.tensor_mul(v_sb, sld, srps)

            if t == R:
                nc.sync.dma_start(v_out, v_sb)
            else:
                w_acc = smp.tile([P, P], BF16, tag="w_acc", bufs=2,
                                 name="w_acc")
                if t == 1:
                    nc.scalar.copy(w_acc, v_sb)
                else:
                    nc.vector.tensor_add(w_acc, w_acc_prev, v_sb)
                w_acc_prev = w_acc
                w_dram = dram.tile([ROW], BF16, tag="w_dram")
                nc.sync.dma_start(
                    w_dram.rearrange("(p q) -> p q", p=P), w_acc)
                wd_b = w_dram.unsqueeze(0)
                wd_v = wd_b.rearrange("x (q c) -> x q c", c=QT)
                for h in (0, 1):
                    sl = slice(h * (QT // 2), (h + 1) * (QT // 2))
                    nc.sync.dma_start(
                        fhalf(w_fo, h),
                        wd_v[:, :, sl].broadcast_to([P, NQT, QT // 2]))


def _build(routing_num: int):
    R = int(routing_num)
    assert R >= 1
    nc = bacc.Bacc(
        "TRN2", target_bir_lowering=False, debug=False, num_devices=CORES)
    uh = nc.dram_tensor("uh", [I_LOC, ROW], F32, kind="ExternalInput")
    v_out = nc.dram_tensor("v_out", [P, P], BF16, kind="ExternalOutput")
    rg = [list(range(CORES))]
    with tile.TileContext(nc) as tc:
        _body(nc, tc, uh.ap(), v_out.ap(), R, rg)
    nc.compile()
    return nc


_CACHE: dict = {}


def _get_nc(routing_num: int):
    R = int(routing_num)
    if R not in _CACHE:
        _CACHE[R] = _build(R)
    return _CACHE[R]


def _shard(u_hat: np.ndarray):
    uh = np.ascontiguousarray(np.asarray(u_hat, dtype=np.float32))
    assert uh.shape == (IN_NODES * OUT_NODES, F_SIZE), uh.shape
    uh = uh.reshape(IN_NODES, ROW)
    return [
        {"uh": np.ascontiguousarray(uh[k * I_LOC:(k + 1) * I_LOC])}
        for k in range(CORES)
    ]


def run(u_hat, routing_num, trace=False):
    nc = _get_nc(routing_num)
    in_maps = _shard(u_hat)
    res = bass_utils.run_bass_kernel_spmd(
        nc, in_maps, core_ids=list(range(CORES)), trace=trace)
    return res


def _unpack(v_pm) -> np.ndarray:
    # [128,128] p-major bf16, p = q*32 + f*2 + hi, free = lo
    # o = q*256 + hi*128 + lo  ->  [1024, 16] f32
    v = np.asarray(v_pm).astype(np.float32).reshape(NQT, F_SIZE, 2, P)
    return np.ascontiguousarray(
        v.transpose(0, 2, 3, 1).reshape(OUT_NODES, F_SIZE))


def kernel(u_hat, routing_num):
    res = run(u_hat, routing_num, trace=False)
    return _unpack(res.results[0]["v_out"])



# revision 2
# speedup vs baseline: 1.0299x; 1.0299x over previous
"""DGL capsule routing layer on 8 trn2 NeuronCores (Bass/Tile) — v5.

Math per iteration (b0 = 0):
    c = softmax(b, axis=out); s = einsum('io,iof->of', c, uh)
    v = squash(s); b += einsum('iof,of->io', uh, v)
Output: final v [OUT, F].

v5 = v4 (f-major bf16 host-side shard, 436us) + running-b reformulation:
    v_t = g_t[o] * s_t  with  g = |s|/(1+|s|^2)  (squash scale per o)
    b_t = b_{t-1} + g_{t-1}[o] * (sum_f uh[i,o,f] * s_{t-1}[o,f])
  so passes >= 2 consume the RAW AllReduce output s directly:
  * s_q broadcast tiles [128, QT] fill per-quarter right after each
    AR_q completes — overlapped with the remaining AR chain, instead
    of a post-squash 4MB w broadcast on the critical path.
  * the post-AR serial work shrinks to the [8,128] g-chain (one Sqrt
    table) + a 256KB g_rep broadcast.
  * b lives as 4 persistent [128,1024] bf16 tiles, updated in place:
    b += g_rep .* tree(uh .* s_bcast);  e = Exp(b) as before.
  * no w accumulation, no w_dram bounce, no srps/v_sb except in the
    final pass (output tail unchanged).
  * pass-1 AR split in 2 contiguous halves {q0,q1}, {q2,q3}: half-A
    s-broadcast overlaps the AR of half B.
  * the previous boundary's g-chain is emitted AFTER block 0's tree in
    the next pass so DVE program order lets block-0 tree muls pace the
    in-flight AR chain (q-mul k waits only AR_k + bcast).

Layout (unchanged from v4): cache col g = f*1024 + o; p-major [128,128]
view of f-major flat: p = f*8 + (o>>7), c = o&127; o-group m = p % 8.
"""

import numpy as np
from contextlib import ExitStack

import ml_dtypes

import concourse.bass as bass
import concourse.mybir as mybir
import concourse.tile as tile
from concourse import bacc
from concourse import bass_utils

F32 = mybir.dt.float32
BF16 = mybir.dt.bfloat16
AF = mybir.ActivationFunctionType
AO = mybir.AluOpType

IN_NODES, OUT_NODES, F_SIZE = 4096, 1024, 16
CORES = 8
I_LOC = IN_NODES // CORES          # 512 in-nodes per core
ROW = OUT_NODES * F_SIZE           # 16384 cache cols per in-node
P = 128
NBLK = I_LOC // P                  # 4 i-blocks per core
O = OUT_NODES
QT = ROW // 4                      # 4096 cols per f-quarter (4 f-planes)
NQ = 4
H = ROW // 2


def _body(nc, tc, uh, v_out, R, rg):
    uh_t = uh.rearrange("(n p) r -> n p r", p=P)   # [NBLK, 128, 16384] bf16

    with ExitStack() as ctx:
        persist = ctx.enter_context(tc.tile_pool(name="persist", bufs=1))
        scp = ctx.enter_context(tc.tile_pool(name="scp", bufs=1))
        smp = ctx.enter_context(tc.tile_pool(name="smp", bufs=1))
        psp = ctx.enter_context(tc.tile_pool(name="psp", bufs=3, space="PSUM"))
        psq = ctx.enter_context(tc.tile_pool(name="psq", bufs=2, space="PSUM"))
        dram = ctx.enter_context(tc.tile_pool(name="dram", bufs=2, space="DRAM"))

        # --- persistent tiles -------------------------------------------
        uhb = [persist.tile([P, ROW], BF16, name=f"uhb{k}", tag=f"uhb{k}")
               for k in range(NBLK)]
        s_qs = b_prev = g_rep = None
        if R > 1:
            s_qs = [persist.tile([P, QT], BF16, name=f"s_q{q}", tag=f"s_q{q}")
                    for q in range(NQ)]
            b_prev = [persist.tile([P, O], BF16, name=f"bp{k}", tag=f"bp{k}")
                      for k in range(NBLK)]
            g_rep = persist.tile([P, O], BF16, name="g_rep", tag="g_rep")
        c0 = persist.tile([P, 1], BF16, name="c0")
        nc.vector.memset(c0, 1.0 / OUT_NODES)
        # squash one-hots in the f-major p-mapping (o-group m = p % 8)
        pidx = np.arange(P)
        m_of_p = pidx % 8
        oh1_d = nc.inline_tensor(
            (m_of_p[:, None] == np.arange(8)[None, :]).astype('bfloat16'),
            name="oh1d")
        oh2_d = nc.inline_tensor(
            (np.arange(8)[:, None] == m_of_p[None, :]).astype('bfloat16'),
            name="oh2d")
        oh1 = persist.tile([P, 8], BF16, name="oh1")
        nc.sync.dma_start(oh1, oh1_d.ap())
        oh2 = persist.tile([8, P], BF16, name="oh2")
        nc.sync.dma_start(oh2, oh2_d.ap())

        def tree_d(blk):
            """d = sum_f uh .* s (tree over quarters, in AR order)."""
            acc = scp.tile([P, QT], BF16, tag="acc", name="acc")
            nc.vector.tensor_mul(acc, uhb[blk][:, 0:QT], s_qs[0])
            for q in (1, 2, 3):
                tmp = scp.tile([P, QT], BF16, tag="tmp", name="tmp")
                nc.vector.tensor_mul(
                    tmp, uhb[blk][:, q * QT:(q + 1) * QT], s_qs[q])
                nc.vector.tensor_add(acc, acc, tmp)
            nc.vector.tensor_add(acc[:, 0:2048], acc[:, 0:2048],
                                 acc[:, 2048:4096])
            d = scp.tile([P, O], BF16, tag="pt", bufs=2, name="d")
            nc.vector.tensor_add(d, acc[:, 0:O], acc[:, O:2 * O])
            return d

        def g_chain(sld_srcs):
            """sc[8,128] = |s|/(1+|s|^2) from the AR result quarters."""
            sld = smp.tile([P, P], BF16, tag="sld", bufs=2, name="sld")
            for src, sl in sld_srcs:
                nc.sync.dma_start(sld[sl, :], src)
            ssq = smp.tile([P, P], BF16, tag="ssq", name="ssq")
            nc.vector.tensor_mul(ssq, sld, sld)
            sqps = psq.tile([8, P], F32, tag="sqps", bufs=1, name="sqps")
            nc.tensor.matmul(sqps, oh1, ssq, start=True, stop=True,
                             skip_group_check=True)
            sq = smp.tile([8, P], BF16, tag="sq", name="sq")
            nc.scalar.copy(sq, sqps)
            y = smp.tile([8, P], BF16, tag="y", name="y")
            nc.scalar.activation(y, sq, AF.Sqrt)
            d1 = smp.tile([8, P], BF16, tag="t1", name="d1")
            nc.vector.tensor_scalar(d1, sq, 1.0, None, AO.add)
            rd = smp.tile([8, P], BF16, tag="ry", name="rd")
            with nc.allow_low_precision(reason="bf16 squash chain"):
                nc.vector.reciprocal(rd, d1)
            sc = smp.tile([8, P], BF16, tag="sq2", name="sc")
            nc.vector.tensor_mul(sc, y, rd)
            return sld, sc

        def gd_b_e2(blk, d, t):
            gd = scp.tile([P, O], BF16, tag="pt", bufs=2, name="gd")
            nc.vector.tensor_mul(gd, d, g_rep)
            if t == 2:
                nc.vector.tensor_copy(b_prev[blk], gd)
            else:
                nc.vector.tensor_add(b_prev[blk], b_prev[blk], gd)
            e2 = smp.tile([P, O], BF16, tag=f"e2_{blk}", name="e2")
            den = smp.tile([P, 1], F32, tag="den", name="den")
            nc.scalar.activation(e2, b_prev[blk], AF.Exp, accum_out=den)
            rinv = smp.tile([P, 1], F32, tag="rinv", name="rinv")
            nc.vector.reciprocal(rinv, den)
            rb = smp.tile([P, 1], BF16, tag=f"rb_{blk}", name="rb")
            nc.vector.tensor_copy(rb, rinv)
            return e2, rb

        pending = None   # sld_srcs of the boundary awaiting its g-chain

        for t in range(1, R + 1):
            final = (t == R)
            if t == 1:
                # two contiguous halves {q0,q1} and {q2,q3}
                ar_hin = [dram.tile([H], BF16, tag=f"arh_in{h}",
                                    name=f"ahi{h}") for h in range(2)]
                ar_hout = [dram.tile([H], BF16, tag=f"arh_out{h}",
                                     name=f"aho{h}") for h in range(2)]
                for fq in range(NQ):
                    sp = slice(fq * QT, (fq + 1) * QT)
                    for blk in range(NBLK):
                        nc.sync.dma_start(uhb[blk][:, sp], uh_t[blk, :, sp])
                    for sub in range(4):
                        ps = psp.tile([1, O], F32, tag="ps1", name="ps",
                                      padded_shape=[P, O])
                        base = fq * QT + sub * O
                        for blk in range(NBLK):
                            for w0 in (0, 512):
                                nc.tensor.matmul(
                                    ps[:, w0:w0 + 512], c0,
                                    uhb[blk][:, base + w0:base + w0 + 512],
                                    start=(blk == 0), stop=(blk == NBLK - 1),
                                    skip_group_check=True)
                        fl = smp.tile([1, O], BF16, tag="bfl", bufs=2,
                                      name="fl")
                        nc.scalar.copy(fl, ps)
                        nc.sync.dma_start(
                            ar_hin[fq // 2][(fq % 2) * QT + sub * O:
                                            (fq % 2) * QT + (sub + 1) * O],
                            fl)
                    if fq % 2 == 1:
                        nc.gpsimd.collective_compute(
                            "AllReduce", AO.add, replica_groups=rg,
                            ins=[ar_hin[fq // 2].opt()],
                            outs=[ar_hout[fq // 2].opt()])
                ar_q = [ar_hout[q // 2][(q % 2) * QT:(q % 2 + 1) * QT]
                        for q in range(NQ)]
                sld_srcs = [(ar_hout[h].rearrange("(p c) -> p c", c=P),
                             slice(h * 64, (h + 1) * 64)) for h in range(2)]
            else:
                # ---- passes >= 2: block-0 tree paces the AR chain, then
                # the DEFERRED g-chain of the previous boundary, then the
                # rest of the blocks.
                d0 = tree_d(0)
                sld, sc = g_chain(pending)
                g_dram = dram.tile([O], BF16, tag="g_dram", name="g_dram")
                nc.sync.dma_start(
                    g_dram.rearrange("(p c) -> p c", p=8), sc)
                nc.sync.dma_start(
                    g_rep, g_dram[None, :].broadcast_to([P, O]))
                rbs = [None] * NBLK
                e2s = [None] * NBLK
                e2s[0], rbs[0] = gd_b_e2(0, d0, t)
                for blk in range(1, NBLK):
                    d = tree_d(blk)
                    e2s[blk], rbs[blk] = gd_b_e2(blk, d, t)

                # ---- s partials: fq-outer so AR_q fires early ----------
                ar_ins = [dram.tile([QT], BF16, tag=f"ar_in{q}",
                                    name=f"ari{q}") for q in range(NQ)]
                ar_outs = [dram.tile([QT], BF16, tag=f"ar_out{q}",
                                     name=f"aro{q}") for q in range(NQ)]
                for fq in range(NQ):
                    for sub in range(4):
                        base = fq * QT + sub * O
                        ps = psp.tile([1, O], F32, tag="ps1", name="ps",
                                      padded_shape=[P, O])
                        for blk in range(NBLK):
                            pt = scp.tile([P, O], BF16, tag="pt", bufs=2,
                                          name="pt")
                            nc.vector.tensor_mul(
                                pt, uhb[blk][:, base:base + O], e2s[blk])
                            for w0 in (0, 512):
                                nc.tensor.matmul(
                                    ps[:, w0:w0 + 512], rbs[blk],
                                    pt[:, w0:w0 + 512],
                                    start=(blk == 0), stop=(blk == NBLK - 1),
                                    skip_group_check=True)
                        fl = smp.tile([1, O], BF16, tag="bfl", bufs=2,
                                      name="fl")
                        nc.scalar.copy(fl, ps)
                        nc.sync.dma_start(ar_ins[fq][sub * O:(sub + 1) * O],
                                          fl)
                    nc.gpsimd.collective_compute(
                        "AllReduce", AO.add, replica_groups=rg,
                        ins=[ar_ins[fq].opt()], outs=[ar_outs[fq].opt()])
                ar_q = ar_outs
                sld_srcs = [(ar_outs[q].rearrange("(p c) -> p c", c=P),
                             slice(q * 32, (q + 1) * 32)) for q in range(NQ)]

            if not final:
                # broadcast raw s quarters as their ARs complete; the
                # g-chain for this boundary is deferred into pass t+1
                for q in range(NQ):
                    nc.sync.dma_start(
                        s_qs[q], ar_q[q][None, :].broadcast_to([P, QT]))
                pending = sld_srcs
            else:
                # output tail: full squash, v = sld .* bcast(sc)
                sld, sc = g_chain(sld_srcs)
                srps = psq.tile([P, P], F32, tag="srps", bufs=1, name="srps")
                nc.tensor.matmul(srps, oh2, sc, start=True, stop=True,
                                 skip_group_check=True)
                v_sb = smp.tile([P, P], BF16, tag="v_sb", name="v_sb")
                nc.vector.tensor_mul(v_sb, sld, srps)
                nc.sync.dma_start(v_out, v_sb)


def _build(routing_num: int):
    R = int(routing_num)
    assert R >= 1
    nc = bacc.Bacc(
        "TRN2", target_bir_lowering=False, debug=False, num_devices=CORES)
    uh = nc.dram_tensor("uh", [I_LOC, ROW], BF16, kind="ExternalInput")
    v_out = nc.dram_tensor("v_out", [P, P], BF16, kind="ExternalOutput")
    rg = [list(range(CORES))]
    with tile.TileContext(nc) as tc:
        _body(nc, tc, uh.ap(), v_out.ap(), R, rg)
    nc.compile()
    return nc


_CACHE: dict = {}


def _get_nc(routing_num: int):
    R = int(routing_num)
    if R not in _CACHE:
        _CACHE[R] = _build(R)
    return _CACHE[R]


def _shard(u_hat: np.ndarray):
    uh = np.asarray(u_hat, dtype=np.float32)
    assert uh.shape == (IN_NODES * OUT_NODES, F_SIZE), uh.shape
    # per core: [512, 1024, 16] -> f-major [512, 16, 1024] bf16
    uh = uh.reshape(IN_NODES, OUT_NODES, F_SIZE)
    out = []
    for k in range(CORES):
        blkk = uh[k * I_LOC:(k + 1) * I_LOC]          # [512, 1024, 16]
        fm = np.ascontiguousarray(blkk.transpose(0, 2, 1)).reshape(I_LOC, ROW)
        out.append({"uh": fm.astype(ml_dtypes.bfloat16)})
    return out


def run(u_hat, routing_num, trace=False):
    nc = _get_nc(routing_num)
    in_maps = _shard(u_hat)
    res = bass_utils.run_bass_kernel_spmd(
        nc, in_maps, core_ids=list(range(CORES)), trace=trace)
    return res


def _unpack(v_pm) -> np.ndarray:
    # [128,128] p-major bf16, p = f*8 + (o>>7), c = o & 127
    v = np.asarray(v_pm).astype(np.float32).reshape(F_SIZE, 8, P)
    return np.ascontiguousarray(
        v.transpose(1, 2, 0).reshape(OUT_NODES, F_SIZE))


def kernel(u_hat, routing_num):
    res = run(u_hat, routing_num, trace=False)
    return _unpack(res.results[0]["v_out"])
